# revision 1
# baseline (speedup 1.0000x reference)
"""Trainium2 Bass kernel for nn_EncoderLayer_88476326298146 (sparse graph attention).

Row-sharded across 8 NeuronCores: core c owns nodes [c*2048, (c+1)*2048) and the
edges targeting them (host-sorted by row into 16 windows of 128 rows, padded to a
fixed per-window count TW). k/v (bf16, [k|v] rows) are replicated via AllGather;
per-edge col features come from dma_gather. Scores use the m=0 softmax (exp is
bounded: score <= max pos_att_bias), segment sums run on the PE via host-built
one-hot matrices, and bvec is folded into Wvec as a 4th row with rel4=[rel,1].
"""
import os
import numpy as np

import concourse.bass as bass
import concourse.bacc as bacc
import concourse.mybir as mybir
import concourse.tile as tile
from concourse.bass_utils import run_bass_kernel_spmd
from concourse.library_config import mlp as mlp_lib
from concourse.masks import make_identity

L, E, SP, C, H, DH, HID = 16384, 131072, 20000, 512, 8, 64, 1024
NCORES = 8
RS = L // NCORES
NW = RS // 128
P = 128
F32 = mybir.dt.float32
BF16 = mybir.dt.bfloat16
I16 = mybir.dt.int16
AF = mybir.ActivationFunctionType
ALU = mybir.AluOpType
AX = mybir.AxisListType

_cache = {}
_SKIP = set(os.environ.get("KSKIP", "").split(","))


def _build(TW):
    NT = TW // P
    nc = bacc.Bacc("TRN2", target_bir_lowering=False, debug=False, num_devices=NCORES)

    x_in = nc.dram_tensor("x_in", [RS, C], F32, kind="ExternalInput")
    w_qkv = nc.dram_tensor("w_qkv", [C, 3 * C], F32, kind="ExternalInput")
    b_qkv = nc.dram_tensor("b_qkv", [P, 3 * C], F32, kind="ExternalInput")
    w_o = nc.dram_tensor("w_o", [C, C], F32, kind="ExternalInput")
    b_o = nc.dram_tensor("b_o", [P, C], F32, kind="ExternalInput")
    w_1 = nc.dram_tensor("w_1", [C, HID], F32, kind="ExternalInput")
    b_1 = nc.dram_tensor("b_1", [P, HID], F32, kind="ExternalInput")
    w_2 = nc.dram_tensor("w_2", [HID, C], F32, kind="ExternalInput")
    b_2 = nc.dram_tensor("b_2", [P, C], F32, kind="ExternalInput")
    w_vec4 = nc.dram_tensor("w_vec4", [32, C], F32, kind="ExternalInput")
    ident_in = nc.dram_tensor("ident_in", [P, P], F32, kind="ExternalInput")
    eps_in = nc.dram_tensor("eps_in", [P, 1], F32, kind="ExternalInput")
    ln_rep = nc.dram_tensor("ln_rep", [4, P, C], F32, kind="ExternalInput")
    eidx = nc.dram_tensor("eidx", [NW, P, TW // 16], I16, kind="ExternalInput")
    geo = nc.dram_tensor("geo", [NW, P, NT, 8], F32, kind="ExternalInput")
    biasA = nc.dram_tensor("biasA", [NW, P, NT, 8], F32, kind="ExternalInput")
    s_e2r = nc.dram_tensor("s_e2r", [NW, P, NT, P], BF16, kind="ExternalInput")
    s_r2e = nc.dram_tensor("s_r2e", [NW, P, NT, P], BF16, kind="ExternalInput")
    y_out = nc.dram_tensor("y_out", [RS, C], F32, kind="ExternalOutput")

    x_t = x_in.ap().rearrange("(m p) n -> p m n", p=P)
    inv_s = 1.0 / float(np.sqrt(DH))

    with tile.TileContext(nc) as tc:
        with tc.tile_pool(name="dram", bufs=1, space="DRAM") as dram, \
             tc.tile_pool(name="const", bufs=1) as const:
            nc.gpsimd.load_library(mlp_lib)

            ident = const.tile([P, P], BF16)
            nc.gpsimd.dma_start(ident[:], ident_in.ap())
            eps_t = const.tile([P, 1], F32)
            nc.sync.dma_start(eps_t[:], eps_in.ap())
            wvec_s = const.tile([32, C], BF16)
            nc.gpsimd.dma_start(wvec_s[:], w_vec4.ap())
            q_sbuf = const.tile([P, NW, C], BF16)
            zt_sbuf = const.tile([P, C // P, RS], BF16)

            kv_shard = dram.tile([RS, 2 * C], BF16)
            if "ag" not in _SKIP:
                kvt = dram.tile([L, 2 * C], BF16, addr_space="Shared")
            else:
                kvt = dram.tile([L, 2 * C], BF16)
            attin_d = dram.tile([RS, C], BF16)
            x2_d = dram.tile([RS, C], F32)
            h_d = dram.tile([RS, HID], BF16)

            # ---------- LN helper ----------
            def ln_phase(src_tiled, g_row, b_row):
                with tc.tile_pool(name="lnp", bufs=2) as wk, \
                     tc.tile_pool(name="lnc", bufs=1) as cst, \
                     tc.tile_pool(name="lntp", bufs=2, space="PSUM") as ptp:
                    gr = cst.tile([P, C], F32, name="lng")
                    nc.sync.dma_start(gr[:], g_row)
                    br = cst.tile([P, C], F32, name="lnb")
                    nc.sync.dma_start(br[:], b_row)
                    for m in range(NW):
                        xt = wk.tile([P, C], F32, tag="ln_x")
                        nc.sync.dma_start(xt[:], src_tiled[:, m, :])
                        sx = wk.tile([P, 1], F32, tag="ln_sx")
                        sq = wk.tile([P, C], F32, tag="ln_sq")
                        sx2 = wk.tile([P, 1], F32, tag="ln_sx2")
                        nc.vector.reduce_sum(sx[:], xt[:], axis=AX.X)
                        nc.scalar.activation(sq[:], xt[:], AF.Square)
                        nc.vector.reduce_sum(sx2[:], sq[:], axis=AX.X)
                        mu = wk.tile([P, 1], F32, tag="ln_mu")
                        nc.vector.tensor_scalar_mul(mu[:], sx[:], 1.0 / C)
                        mu2 = wk.tile([P, 1], F32, tag="ln_mu2")
                        nc.vector.tensor_mul(mu2[:], mu[:], mu[:])
                        var = wk.tile([P, 1], F32, tag="ln_var")
                        nc.vector.scalar_tensor_tensor(
                            out=var[:], in0=sx2[:], scalar=1.0 / C, in1=mu2[:],
                            op0=ALU.mult, op1=ALU.subtract)
                        sd = wk.tile([P, 1], F32, tag="ln_sd")
                        nc.scalar.activation(sd[:], var[:], AF.Sqrt, bias=eps_t[:], scale=1.0)
                        rs_ = wk.tile([P, 1], F32, tag="ln_rs")
                        nc.vector.reciprocal(rs_[:], sd[:])
                        nmr = wk.tile([P, 1], F32, tag="ln_nmr")
                        nc.vector.scalar_tensor_tensor(
                            out=nmr[:], in0=mu[:], scalar=-1.0, in1=rs_[:],
                            op0=ALU.mult, op1=ALU.mult)
                        zf = wk.tile([P, C], F32, tag="ln_z")
                        nc.scalar.activation(zf[:], xt[:], AF.Identity, bias=nmr[:], scale=rs_[:])
                        nc.vector.tensor_mul(zf[:], zf[:], gr[:])
                        zb = wk.tile([P, C], BF16, tag="ln_zb")
                        nc.vector.tensor_add(zb[:], zf[:], br[:])
                        for c4 in range(C // P):
                            tp = ptp.tile([P, P], BF16, tag="tp")
                            nc.tensor.transpose(tp[:], zb[:, c4 * P:(c4 + 1) * P], ident[:])
                            nc.vector.tensor_copy(zt_sbuf[:, c4, m * P:(m + 1) * P], tp[:])

            # ============ P1: LN1 -> zT ============
            if "ln1" not in _SKIP:
                ln_phase(x_t, ln_rep.ap()[0], ln_rep.ap()[1])

            # ============ P2: QKV ============
            if "p2" not in _SKIP:
             with tc.tile_pool(name="p2", bufs=2) as wk, \
                 tc.tile_pool(name="p2c", bufs=1) as cst, \
                 tc.tile_pool(name="p2ps", bufs=2, space="PSUM") as pps:
                wqkv_s = cst.tile([P, C // P, 3 * C], BF16, name="wqkv")
                nc.gpsimd.dma_start(wqkv_s[:], w_qkv.ap().rearrange("(ko p) n -> p ko n", p=P))
                bqkv_s = cst.tile([P, 3 * C], F32, name="bqkv")
                nc.sync.dma_start(bqkv_s[:], b_qkv.ap())
                kv_sh_t = kv_shard[:].rearrange("(m p) n -> p m n", p=P)
                for m in range(NW):
                    for nb in range(3):
                        ps = pps.tile([P, 512], F32, tag="ps")
                        for ko in range(C // P):
                            nc.tensor.matmul(
                                ps[:],
                                lhsT=zt_sbuf[:, ko, m * P:(m + 1) * P],
                                rhs=wqkv_s[:, ko, nb * 512:(nb + 1) * 512],
                                start=(ko == 0), stop=(ko == C // P - 1))
                        if nb == 0:
                            nc.vector.scalar_tensor_tensor(
                                out=q_sbuf[:, m, :], in0=ps[:], scalar=1.0,
                                in1=bqkv_s[:, 0:512], op0=ALU.mult, op1=ALU.add)
                        else:
                            kvb = wk.tile([P, 512], BF16, tag="kvb")
                            nc.vector.scalar_tensor_tensor(
                                out=kvb[:], in0=ps[:], scalar=1.0,
                                in1=bqkv_s[:, nb * 512:(nb + 1) * 512],
                                op0=ALU.mult, op1=ALU.add)
                            nc.sync.dma_start(kv_sh_t[:, m, (nb - 1) * 512:nb * 512], kvb[:])

            # ============ P3: AllGather ============
            if "ag" not in _SKIP:
                nc.gpsimd.collective_compute(
                    "AllGather", ALU.bypass, replica_groups=[list(range(NCORES))],
                    ins=[kv_shard[:].opt()], outs=[kvt[:].opt()])

            # ============ P4: edge windows ============
            if "edge" not in _SKIP:
                with tc.tile_pool(name="big", bufs=2) as big, \
                   tc.tile_pool(name="ew", bufs=3) as wk, \
                   tc.tile_pool(name="pqe", bufs=1, space="PSUM") as pqe, \
                   tc.tile_pool(name="ppsW", bufs=1, space="PSUM") as ppsW, \
                   tc.tile_pool(name="ptp2", bufs=1, space="PSUM") as ptp2:
                  for w in range(NW):
                      idx_t = big.tile([P, TW // 16], I16, tag="idx")
                      nc.gpsimd.dma_start(idx_t[:], eidx.ap()[w])
                      kv_g = big.tile([P, NT, 2 * C], BF16, tag="kv")
                      nc.gpsimd.dma_gather(
                          out_ap=kv_g[:], in_ap=kvt[:], idxs_ap=idx_t[:],
                          num_idxs=TW, num_idxs_reg=TW, elem_size=2 * C,
                          single_packet=False)
                      s1_t = big.tile([P, NT, P], BF16, tag="s1")
                      nc.sync.dma_start(s1_t[:], s_e2r.ap()[w])
                      s2_t = big.tile([P, NT, P], BF16, tag="s2")
                      nc.sync.dma_start(s2_t[:], s_r2e.ap()[w])
                      geo_t = big.tile([P, NT, 8], F32, tag="geo")
                      nc.sync.dma_start(geo_t[:], geo.ap()[w])
                      bias_t = big.tile([P, NT, 8], F32, tag="bias")
                      nc.sync.dma_start(bias_t[:], biasA.ap()[w])

                      psW = ppsW.tile([P, 552], F32, tag="psW")
                      for t0 in range(0, NT, 4):
                          tb = min(4, NT - t0)
                          ke = kv_g[:, t0:t0 + tb, 0:C]
                          ve = kv_g[:, t0:t0 + tb, C:2 * C]
                          qe_ps = pqe.tile([P, 4, C], F32, tag="qe")
                          for d_ in range(tb):
                              nc.tensor.matmul(qe_ps[:, d_, :], lhsT=s2_t[:, t0 + d_, :],
                                               rhs=q_sbuf[:, w, :], start=True, stop=True)
                          diff = wk.tile([P, 4, C], BF16, tag="diff")
                          nc.vector.scalar_tensor_tensor(
                              out=diff[:, 0:tb, :], in0=qe_ps[:, 0:tb, :], scalar=1.0,
                              in1=ke, op0=ALU.mult, op1=ALU.subtract)
                          dsq = wk.tile([P, 4, C], BF16, tag="dsq")
                          nc.scalar.activation(dsq[:, 0:tb, :], diff[:, 0:tb, :], AF.Square)
                          s8 = wk.tile([P, 4, H], F32, tag="s8")
                          nc.vector.reduce_sum(
                              s8[:, 0:tb, :],
                              dsq[:, 0:tb, :].rearrange("p t (h d) -> p t h d", h=H),
                              axis=AX.X)
                          sc = wk.tile([P, 4, H], F32, tag="sc")
                          nc.vector.scalar_tensor_tensor(
                              out=sc[:, 0:tb, :], in0=s8[:, 0:tb, :], scalar=-inv_s,
                              in1=bias_t[:, t0:t0 + tb, :], op0=ALU.mult, op1=ALU.add)
                          aux = wk.tile([P, 4, 40], BF16, tag="aux")
                          nc.scalar.activation(aux[:, 0:tb, 0:8], sc[:, 0:tb, :], AF.Exp)
                          rd = wk.tile([P, 4, 1], F32, tag="rd")
                          nc.vector.reciprocal(rd[:, 0:tb, :], geo_t[:, t0:t0 + tb, 3:4])
                          d4 = wk.tile([P, 4, 4], F32, tag="d4")
                          nc.vector.tensor_sub(d4[:, 0:tb, :], geo_t[:, t0:t0 + tb, 0:4],
                                               geo_t[:, t0:t0 + tb, 4:8])
                          rel = wk.tile([P, 4, 4], F32, tag="rel")
                          nc.vector.tensor_mul(
                              rel[:, 0:tb, :], d4[:, 0:tb, :],
                              rd[:, 0:tb, :].broadcast_to([P, tb, 4]))
                          nc.vector.tensor_mul(
                              aux[:, 0:tb, 8:40].rearrange("p t (h r) -> p t h r", h=H),
                              aux[:, 0:tb, 0:8].unsqueeze(3).broadcast_to([P, tb, H, 4]),
                              rel[:, 0:tb, :].unsqueeze(2).broadcast_to([P, tb, H, 4]))
                          pev = wk.tile([P, 4, C], BF16, tag="pev")
                          nc.vector.tensor_mul(
                              pev[:, 0:tb, :].rearrange("p t (h d) -> p t h d", h=H),
                              aux[:, 0:tb, 0:8].unsqueeze(3).broadcast_to([P, tb, H, DH]),
                              ve.rearrange("p t (h d) -> p t h d", h=H))
                          for d_ in range(tb):
                              t = t0 + d_
                              nc.tensor.matmul(psW[:, 0:512], lhsT=s1_t[:, t, :],
                                               rhs=pev[:, d_, :],
                                               start=(t == 0), stop=(t == NT - 1))
                              nc.tensor.matmul(psW[:, 512:552], lhsT=s1_t[:, t, :],
                                               rhs=aux[:, d_, :],
                                               start=(t == 0), stop=(t == NT - 1))

                      den = wk.tile([P, H], F32, tag="den")
                      nc.vector.tensor_scalar_max(den[:], psW[:, 512:520], 1e-30)
                      rden = wk.tile([P, H], F32, tag="rden")
                      nc.vector.reciprocal(rden[:], den[:])
                      outn = wk.tile([P, C], F32, tag="outn")
                      nc.vector.tensor_mul(
                          outn[:].rearrange("p (h d) -> p h d", h=H),
                          psW[:, 0:512].rearrange("p (h d) -> p h d", h=H),
                          rden[:].unsqueeze(2).broadcast_to([P, H, DH]))
                      an = wk.tile([P, 32], BF16, tag="an")
                      nc.vector.tensor_mul(
                          an[:].rearrange("p (h r) -> p h r", h=H),
                          psW[:, 520:552].rearrange("p (h r) -> p h r", h=H),
                          rden[:].unsqueeze(2).broadcast_to([P, H, 4]))
                      an_tp = ptp2.tile([32, P], BF16, tag="tp2")
                      nc.tensor.transpose(an_tp[:], an[:], ident[:])
                      an_ts = wk.tile([32, P], BF16, tag="an_ts")
                      nc.vector.tensor_copy(an_ts[:], an_tp[:])
                      out2 = ptp2.tile([P, C], F32, tag="out2")
                      nc.tensor.matmul(out2[:], lhsT=an_ts[:], rhs=wvec_s[:],
                                       start=True, stop=True)
                      attin = wk.tile([P, C], BF16, tag="attin")
                      nc.vector.tensor_add(attin[:], outn[:], out2[:])
                      nc.sync.dma_start(
                          attin_d[:].rearrange("(m p) n -> p m n", p=P)[:, w, :], attin[:])

            # ============ P5: x2 = x + attin@Wo + bo ============
            if "p5" not in _SKIP:
             with tc.tile_pool(name="p5", bufs=2) as wk, \
                 tc.tile_pool(name="p5c", bufs=1) as cst, \
                 tc.tile_pool(name="p5tp", bufs=2, space="PSUM") as ptp, \
                 tc.tile_pool(name="p5ps", bufs=2, space="PSUM") as pps:
                wo_s = cst.tile([P, C // P, C], BF16, name="wo")
                nc.gpsimd.dma_start(wo_s[:], w_o.ap().rearrange("(ko p) n -> p ko n", p=P))
                bo_s = cst.tile([P, C], F32, name="bo")
                nc.sync.dma_start(bo_s[:], b_o.ap())
                attin_t = attin_d[:].rearrange("(m p) n -> p m n", p=P)
                x2_t = x2_d[:].rearrange("(m p) n -> p m n", p=P)
                for m in range(NW):
                    at_b = wk.tile([P, C], BF16, tag="at_b")
                    nc.sync.dma_start(at_b[:], attin_t[:, m, :])
                    at_T = wk.tile([P, C // P, P], BF16, tag="at_T")
                    for c4 in range(C // P):
                        tp = ptp.tile([P, P], BF16, tag="tp")
                        nc.tensor.transpose(tp[:], at_b[:, c4 * P:(c4 + 1) * P], ident[:])
                        nc.vector.tensor_copy(at_T[:, c4, :], tp[:])
                    ps = pps.tile([P, C], F32, tag="ps")
                    for ko in range(C // P):
                        nc.tensor.matmul(ps[:], lhsT=at_T[:, ko, :], rhs=wo_s[:, ko, :],
                                         start=(ko == 0), stop=(ko == C // P - 1))
                    xt = wk.tile([P, C], F32, tag="x")
                    nc.sync.dma_start(xt[:], x_t[:, m, :])
                    x2t = wk.tile([P, C], F32, tag="x2")
                    nc.vector.scalar_tensor_tensor(
                        out=x2t[:], in0=ps[:], scalar=1.0, in1=bo_s[:],
                        op0=ALU.mult, op1=ALU.add)
                    nc.vector.tensor_add(x2t[:], x2t[:], xt[:])
                    nc.sync.dma_start(x2_t[:, m, :], x2t[:])

            # ============ P6: LN2 -> zT ============
            if "p6" not in _SKIP:
             ln_phase(x2_d[:].rearrange("(m p) n -> p m n", p=P),
                     ln_rep.ap()[2], ln_rep.ap()[3])

            # ============ P7: FFN1 ============
            if "p7" not in _SKIP:
             with tc.tile_pool(name="p7", bufs=2) as wk, \
                 tc.tile_pool(name="p7c", bufs=1) as cst, \
                 tc.tile_pool(name="p7ps", bufs=2, space="PSUM") as pps:
                w1_s = cst.tile([P, C // P, HID], BF16, name="w1")
                nc.gpsimd.dma_start(w1_s[:], w_1.ap().rearrange("(ko p) n -> p ko n", p=P))
                b1_s = cst.tile([P, HID], F32, name="b1")
                nc.sync.dma_start(b1_s[:], b_1.ap())
                h_t = h_d[:].rearrange("(m p) n -> p m n", p=P)
                for m in range(NW):
                    for nb in range(HID // 512):
                        ps = pps.tile([P, 512], F32, tag="ps")
                        for ko in range(C // P):
                            nc.tensor.matmul(
                                ps[:], lhsT=zt_sbuf[:, ko, m * P:(m + 1) * P],
                                rhs=w1_s[:, ko, nb * 512:(nb + 1) * 512],
                                start=(ko == 0), stop=(ko == C // P - 1))
                        hb = wk.tile([P, 512], F32, tag="hb")
                        nc.vector.scalar_tensor_tensor(
                            out=hb[:], in0=ps[:], scalar=1.0,
                            in1=b1_s[:, nb * 512:(nb + 1) * 512],
                            op0=ALU.mult, op1=ALU.add)
                        hg = wk.tile([P, 512], BF16, tag="hg")
                        nc.scalar.activation(hg[:], hb[:], AF.Gelu_apprx_tanh)
                        nc.sync.dma_start(h_t[:, m, nb * 512:(nb + 1) * 512], hg[:])

            # ============ P8: y = h@W2 + b2 + x2 ============
            if "p8" not in _SKIP:
             with tc.tile_pool(name="p8", bufs=2) as wk, \
                 tc.tile_pool(name="p8c", bufs=1) as cst, \
                 tc.tile_pool(name="p8tp", bufs=2, space="PSUM") as ptp, \
                 tc.tile_pool(name="p8ps", bufs=2, space="PSUM") as pps:
                w2_s = cst.tile([P, HID // P, C], BF16, name="w2")
                nc.gpsimd.dma_start(w2_s[:], w_2.ap().rearrange("(ko p) n -> p ko n", p=P))
                b2_s = cst.tile([P, C], F32, name="b2")
                nc.sync.dma_start(b2_s[:], b_2.ap())
                h_t = h_d[:].rearrange("(m p) n -> p m n", p=P)
                x2_t = x2_d[:].rearrange("(m p) n -> p m n", p=P)
                y_t = y_out.ap().rearrange("(m p) n -> p m n", p=P)
                for m in range(NW):
                    hb = wk.tile([P, HID], BF16, tag="hb")
                    nc.sync.dma_start(hb[:], h_t[:, m, :])
                    h_T = wk.tile([P, HID // P, P], BF16, tag="hT")
                    for c8 in range(HID // P):
                        tp = ptp.tile([P, P], BF16, tag="tp")
                        nc.tensor.transpose(tp[:], hb[:, c8 * P:(c8 + 1) * P], ident[:])
                        nc.vector.tensor_copy(h_T[:, c8, :], tp[:])
                    ps = pps.tile([P, C], F32, tag="ps")
                    for ko in range(HID // P):
                        nc.tensor.matmul(ps[:], lhsT=h_T[:, ko, :], rhs=w2_s[:, ko, :],
                                         start=(ko == 0), stop=(ko == HID // P - 1))
                    x2t = wk.tile([P, C], F32, tag="x2")
                    nc.sync.dma_start(x2t[:], x2_t[:, m, :])
                    yt = wk.tile([P, C], F32, tag="y")
                    nc.vector.scalar_tensor_tensor(
                        out=yt[:], in0=ps[:], scalar=1.0, in1=b2_s[:],
                        op0=ALU.mult, op1=ALU.add)
                    nc.vector.tensor_add(yt[:], yt[:], x2t[:])
                    nc.sync.dma_start(y_t[:, m, :], yt[:])

    nc.compile()
    return nc


def _prep(inputs):
    row = np.asarray(inputs["row_index"]).astype(np.int64).ravel()
    col = np.asarray(inputs["col_index"]).astype(np.int64).ravel()
    tcol = np.asarray(inputs["to_col_index"]).astype(np.int64).ravel()
    bias = np.asarray(inputs["pos_att_bias"], dtype=np.float32)
    dist = np.asarray(inputs["dist"], dtype=np.float32).ravel()
    pos = np.asarray(inputs["pos"], dtype=np.float32)
    cpos = np.asarray(inputs["col_pos"], dtype=np.float32)

    order = np.argsort(row, kind="stable")
    rs_, cs_, ts_ = row[order], col[order], tcol[order]
    win = rs_ // P
    counts = np.bincount(win, minlength=L // P)
    TW = int(np.ceil(max(int(counts.max()), 1) / P) * P)
    NT = TW // P
    starts = np.zeros(L // P + 1, np.int64)
    np.cumsum(counts, out=starts[1:])

    eidx_h = np.zeros((NCORES, NW, P, TW // 16), np.int16)
    geo_h = np.zeros((NCORES, NW, P, NT, 8), np.float32)
    geo_h[..., 3] = 1.0  # pad: dist slot 1 -> rel4 = [0,0,0,1]
    bias_h = np.full((NCORES, NW, P, NT, 8), -1e4, np.float32)
    s1_h = np.zeros((NCORES, NW, P, NT, P), np.float32)
    s2_h = np.zeros((NCORES, NW, P, NT, P), np.float32)

    for gw in range(L // P):
        c, w = divmod(gw, NW)
        s, e = int(starts[gw]), int(starts[gw + 1])
        n = e - s
        if n == 0:
            continue
        ecols = cs_[s:e]
        erows = (rs_[s:e] - gw * P).astype(np.int64)
        eo = order[s:e]
        j = np.arange(n)
        wrap = np.zeros((16, TW // 16), np.int16)
        wrap[j % 16, j // 16] = ecols.astype(np.int16)
        eidx_h[c, w] = np.tile(wrap, (8, 1))
        t_of = j // P
        e_of = j % P
        geo_h[c, w, e_of, t_of, 0:3] = cpos[ts_[s:e]]
        geo_h[c, w, e_of, t_of, 3] = dist[eo]
        geo_h[c, w, e_of, t_of, 4:7] = pos[rs_[s:e]]
        bias_h[c, w, e_of, t_of, :] = bias[eo]
        s1_h[c, w, e_of, t_of, erows] = 1.0
        s2_h[c, w, erows, t_of, e_of] = 1.0

    import ml_dtypes
    return (TW, eidx_h, geo_h, bias_h,
            s1_h.astype(ml_dtypes.bfloat16), s2_h.astype(ml_dtypes.bfloat16))


def kernel(**inputs):
    x = np.asarray(inputs["x"], dtype=np.float32)
    TW, eidx_h, geo_h, bias_h, s1_h, s2_h = _prep(inputs)
    if TW not in _cache:
        _cache[TW] = _build(TW)
    nc = _cache[TW]

    f32 = lambda k: np.asarray(inputs[k], np.float32)
    rep = lambda v: np.ascontiguousarray(np.broadcast_to(v[None, :], (P, v.shape[0])))
    w_qkv = np.concatenate([f32("Wq"), f32("Wk"), f32("Wv")], axis=1)
    b_qkv = rep(np.concatenate([f32("bq"), f32("bk"), f32("bv")]))
    wv4 = np.concatenate([f32("Wvec"), f32("bvec")[None, :]], axis=0)
    w_vec4 = np.zeros((32, C), np.float32)
    for h in range(H):
        w_vec4[4 * h:4 * h + 4, h * DH:(h + 1) * DH] = wv4[:, h * DH:(h + 1) * DH]
    ln_rep = np.stack([rep(f32("ln1_g")), rep(f32("ln1_b")),
                       rep(f32("ln2_g")), rep(f32("ln2_b"))])

    in_maps = []
    for c in range(NCORES):
        in_maps.append(dict(
            x_in=np.ascontiguousarray(x[c * RS:(c + 1) * RS]),
            w_qkv=w_qkv, b_qkv=b_qkv,
            w_o=f32("Wo"), b_o=rep(f32("bo")),
            w_1=f32("W1"), b_1=rep(f32("b1")),
            w_2=f32("W2"), b_2=rep(f32("b2")),
            w_vec4=w_vec4, ln_rep=ln_rep, ident_in=np.eye(P, dtype=np.float32),
            eps_in=np.full((P, 1), 1e-5, np.float32),
            eidx=eidx_h[c], geo=geo_h[c], biasA=bias_h[c],
            s_e2r=s1_h[c], s_r2e=s2_h[c],
        ))
    _last["nc"] = nc
    _last["in_maps"] = in_maps
    res = run_bass_kernel_spmd(nc, in_maps, list(range(NCORES)))
    y = np.concatenate([res.results[c]["y_out"] for c in range(NCORES)], axis=0)
    return np.asarray(y, np.float32)


_last = {}

